# revision 16
# baseline (speedup 1.0000x reference)
"""Trainium2 Bass kernel for CustomMultiHeadAttention.

Problem: B=2, S=2048, D=2048, H=16 heads of Hd=128; y = MHA(q, k, v) with
torch-Linear-style projections (x @ W.T + b) and out projection.

Sharding (8 NeuronCores): data-parallel over batch (2 groups) x tensor-
parallel over heads (4 heads = 512 channels per core). Each core computes
its 4 heads' Q/K/V projections, attention, and a column-sharded partial of
the out projection; the host sums the 4 partials per batch and adds bo.

Per-core device program (all matmuls bf16, K=128 contraction, N=512):
  - host passes x.T and W.T contiguous so no transposes are needed on-chip
  - Q^T, K^T computed channels-major [512c, S]; V sequence-major [S, 512e]
  - scores computed transposed: S^T[t,s] = K^T(:,t)^T Q^T(:,s); exp on ACT
    (scale 1/sqrt(128) folded in); softmax denominator l[s] built by DVE-
    accumulating exp tiles over t-chunks then one ones-column matmul
    (f32r); O^T = P~V accumulated in PSUM; normalized by 1/l broadcast via
    a K=1 ones-row matmul; V bias folded in after normalization
    (sum_t (V+bv) P~ = P~V + bv*l  =>  O/l + bv).
  - partial out-proj Z^T[d,s] = Wo_loc^T O^T accumulated over 512 local
    channels, emitted bf16.
"""

import math

import numpy as np
import ml_dtypes

B = 2
S = 2048
D = 2048
HD = 128          # head dim
TP = 4            # head-group (tensor-parallel) factor
CL = D // TP      # 512 local channels = 4 heads per core
NCORES = 8

_NC = None


def _build_nc(s=S, d=D, cl=CL):
    """Build the per-core Bass program (SPMD: same program, 8 cores)."""
    from contextlib import ExitStack

    import concourse.bass as bass
    import concourse.mybir as mybir
    import concourse.tile as tile
    from concourse import bacc

    f32 = mybir.dt.float32
    f32r = mybir.dt.float32r
    bf16 = mybir.dt.bfloat16
    Exp = mybir.ActivationFunctionType.Exp

    SBW = 512                 # s-block width (matmul moving free dim)
    NSB = s // SBW            # s blocks
    NTB = s // 128            # t blocks (key/value seq chunks)
    KC = d // 128             # contraction chunks over model dim
    NH = cl // HD             # local heads
    SCALE = 1.0 / math.sqrt(HD)

    nc = bacc.Bacc("TRN2", target_bir_lowering=False, debug=False)

    xqT = nc.dram_tensor("xqT", [d, s], bf16, kind="ExternalInput").ap()
    xkT = nc.dram_tensor("xkT", [d, s], bf16, kind="ExternalInput").ap()
    xvT = nc.dram_tensor("xvT", [d, s], bf16, kind="ExternalInput").ap()
    wqT = nc.dram_tensor("wqT", [d, cl], bf16, kind="ExternalInput").ap()
    wkT = nc.dram_tensor("wkT", [d, cl], bf16, kind="ExternalInput").ap()
    wvT = nc.dram_tensor("wvT", [d, cl], bf16, kind="ExternalInput").ap()
    woT = nc.dram_tensor("woT", [cl, d], bf16, kind="ExternalInput").ap()
    bq = nc.dram_tensor("bq", [cl], f32, kind="ExternalInput").ap()
    bk = nc.dram_tensor("bk", [cl], f32, kind="ExternalInput").ap()
    bv = nc.dram_tensor("bv", [cl], f32, kind="ExternalInput").ap()
    zT = nc.dram_tensor("zT", [d, s], bf16, kind="ExternalOutput").ap()

    with tile.TileContext(nc) as tc, ExitStack() as ctx:
        const = ctx.enter_context(tc.tile_pool(name="const", bufs=1))
        qkv = ctx.enter_context(tc.tile_pool(name="qkv", bufs=1))
        wop = ctx.enter_context(tc.tile_pool(name="wop", bufs=1))
        ps_mm = ctx.enter_context(tc.tile_pool(name="ps_mm", bufs=2, space="PSUM"))
        ps_ot = ctx.enter_context(tc.tile_pool(name="ps_ot", bufs=2, space="PSUM"))
        ps_z = ctx.enter_context(tc.tile_pool(name="ps_z", bufs=2, space="PSUM"))
        ps_l = ctx.enter_context(tc.tile_pool(name="ps_l", bufs=2, space="PSUM"))

        # f32r: reduced-precision fp32 matmul inputs (full PE rate at N>=256).
        # The BIR verifier requires f32r matmul operands to be *written* as
        # f32r, and Memset can't emit f32r — so memset f32 and hop via a
        # DVE copy.
        ones_f = const.tile([128, 1], f32, tag="ones_f")
        nc.vector.memset(ones_f, 1.0)
        ones_col = const.tile([128, 1], f32r, tag="ones_col")
        nc.vector.tensor_copy(ones_col, ones_f)
        ones_rf = const.tile([1, 128], f32, tag="ones_rf")
        nc.vector.memset(ones_rf, 1.0)
        ones_row = const.tile([1, 128], f32r, tag="ones_row")
        nc.vector.tensor_copy(ones_row, ones_rf)
        # Biases land via DMA into *_dma, then hop to *_sb on the DVE. The
        # hop pins the DMA wait on a TensorCopy; downstream tensor_scalar
        # ops (TS struct: single sync-wait slot in codegen) then only carry
        # their PE wait.
        def load_bias(b_dram, nm):
            b_dma = const.tile([128, NH], f32, tag=f"{nm}d", name=f"{nm}d")
            nc.sync.dma_start(b_dma, b_dram.rearrange("(m p) -> p m", p=128))
            b_sb = const.tile([128, NH], f32, tag=nm, name=nm)
            nc.vector.tensor_copy(b_sb, b_dma)
            return b_sb

        bq_sb = load_bias(bq, "bq")
        bk_sb = load_bias(bk, "bk")
        bv_sb = load_bias(bv, "bv")

        wo_sb = wop.tile([128, NH, d], bf16, tag="wo")
        nc.sync.dma_start(wo_sb, woT.rearrange("(k p) m -> p k m", p=128))

        # persistent activation tiles
        qt = [qkv.tile([128, s], bf16, tag=f"qt{h}", name=f"qt{h}") for h in range(NH)]
        kt = [qkv.tile([128, s], bf16, tag=f"kt{h}", name=f"kt{h}") for h in range(NH)]
        vt = [qkv.tile([128, cl], bf16, tag=f"vt{t}", name=f"vt{t}") for t in range(NTB)]
        ot = [qkv.tile([128, s], bf16, tag=f"ot{h}", name=f"ot{h}") for h in range(NH)]

        # ---- Phase A: projections -------------------------------------
        with tc.tile_pool(name="wqkv", bufs=1) as wp, \
             tc.tile_pool(name="panels", bufs=2) as panels:

            wq_sb = wp.tile([128, KC, cl], bf16, tag="wq")
            nc.sync.dma_start(wq_sb, wqT.rearrange("(k p) m -> p k m", p=128))
            wk_sb = wp.tile([128, KC, cl], bf16, tag="wk")
            nc.sync.dma_start(wk_sb, wkT.rearrange("(k p) m -> p k m", p=128))
            wv_sb = wp.tile([128, KC, cl], bf16, tag="wv")
            nc.sync.dma_start(wv_sb, wvT.rearrange("(k p) m -> p k m", p=128))

            def qk_proj(x_dram, w_sb, b_sb, outs):
                # outs[m][c, s] = sum_d W[m*128+c, d] x[s, d]  (+ bias)
                for n in range(NSB):
                    xp = panels.tile([128, KC, SBW], bf16, tag="xpanel",
                                     name=f"xp{n}")
                    nc.sync.dma_start(
                        xp, x_dram[:, n * SBW:(n + 1) * SBW]
                        .rearrange("(k p) s -> p k s", p=128))
                    for m in range(NH):
                        ps = ps_mm.tile([128, SBW], f32, tag="mm", name="ps_proj")
                        for k in range(KC):
                            nc.tensor.matmul(
                                ps, lhsT=w_sb[:, k, m * 128:(m + 1) * 128],
                                rhs=xp[:, k, :],
                                start=(k == 0), stop=(k == KC - 1))
                        nc.vector.tensor_scalar_add(
                            outs[m][:, n * SBW:(n + 1) * SBW], ps, b_sb[:, m:m + 1])

            qk_proj(xqT, wq_sb, bq_sb, qt)
            qk_proj(xkT, wk_sb, bk_sb, kt)

            # V sequence-major: vt[t][tt, e] = sum_d x[t*128+tt, d] Wv[e, d]
            for n in range(NSB):
                xp = panels.tile([128, KC, SBW], bf16, tag="xpanel",
                                 name=f"xpv{n}")
                nc.sync.dma_start(
                    xp, xvT[:, n * SBW:(n + 1) * SBW]
                    .rearrange("(k p) s -> p k s", p=128))
                for tsub in range(SBW // 128):
                    t = n * (SBW // 128) + tsub
                    ps = ps_mm.tile([128, cl], f32, tag="mm", name="ps_v")
                    for k in range(KC):
                        nc.tensor.matmul(
                            ps, lhsT=xp[:, k, tsub * 128:(tsub + 1) * 128],
                            rhs=wv_sb[:, k, :],
                            start=(k == 0), stop=(k == KC - 1))
                    nc.vector.tensor_copy(vt[t], ps)

        # ---- Phase B: attention per (head, s-block) --------------------
        with tc.tile_pool(name="ptiles", bufs=2 * NTB) as ppool, \
             tc.tile_pool(name="accp", bufs=2) as accp, \
             tc.tile_pool(name="small", bufs=2) as small:

            for h in range(NH):
                for sb in range(NSB):
                    ssl = slice(sb * SBW, (sb + 1) * SBW)
                    acc = accp.tile([128, SBW], f32r, tag="acc", name="acc")
                    ptiles = []
                    for tb in range(NTB):
                        ps = ps_mm.tile([128, SBW], f32, tag="mm", name="ps_sc")
                        nc.tensor.matmul(
                            ps, lhsT=kt[h][:, tb * 128:(tb + 1) * 128],
                            rhs=qt[h][:, ssl], start=True, stop=True)
                        p = ppool.tile([128, SBW], bf16, tag="p", name="ptile")
                        nc.scalar.activation(p, ps, Exp, scale=SCALE)
                        if tb == 0:
                            nc.vector.tensor_copy(acc, p)
                        else:
                            nc.vector.tensor_add(acc, acc, p)
                        ptiles.append(p)

                    ops_ = ps_ot.tile([128, SBW], f32, tag="ot", name="ps_pv")
                    for tb in range(NTB):
                        nc.tensor.matmul(
                            ops_, lhsT=vt[tb][:, h * 128:(h + 1) * 128],
                            rhs=ptiles[tb],
                            start=(tb == 0), stop=(tb == NTB - 1))

                    # l[s] = sum_t P~ via ones-matmul on the DVE-accumulated
                    # acc; broadcast l to 128 partitions with a K=1 matmul,
                    # THEN take the reciprocal full-lane (a [1,512] DVE
                    # reciprocal runs on one lane: measured 3.3us vs ~0.6us
                    # for the [128,512] form).
                    lps = ps_l.tile([1, SBW], f32, tag="lrb", name="ps_l")
                    nc.tensor.matmul(lps, lhsT=ones_col, rhs=acc,
                                     start=True, stop=True)
                    l_sb = small.tile([1, SBW], f32r, tag="l_sb", name="l_sb")
                    nc.vector.tensor_copy(l_sb, lps)
                    rb = ps_l.tile([128, SBW], f32, tag="lrb", name="ps_rb")
                    nc.tensor.matmul(rb, lhsT=ones_row, rhs=l_sb,
                                     start=True, stop=True)
                    rb_sb = small.tile([128, SBW], f32, tag="rb", name="rb_sb")
                    nc.vector.reciprocal(rb_sb, rb)
                    osl = ot[h][:, ssl]
                    nc.vector.tensor_mul(osl, ops_, rb_sb)
                    nc.vector.tensor_scalar_add(osl, osl, bv_sb[:, h:h + 1])

        # ---- Phase C: partial out-projection ---------------------------
        with tc.tile_pool(name="zout", bufs=3) as zpool:
            for dd in range(KC):
                for sb in range(NSB):
                    ps = ps_z.tile([128, SBW], f32, tag="z", name="ps_z")
                    for eb in range(NH):
                        nc.tensor.matmul(
                            ps, lhsT=wo_sb[:, eb, dd * 128:(dd + 1) * 128],
                            rhs=ot[eb][:, sb * SBW:(sb + 1) * SBW],
                            start=(eb == 0), stop=(eb == NH - 1))
                    zt = zpool.tile([128, SBW], bf16, tag="z", name="z_sb")
                    nc.vector.tensor_copy(zt, ps)
                    nc.sync.dma_start(
                        zT[dd * 128:(dd + 1) * 128, sb * SBW:(sb + 1) * SBW], zt)

    nc.compile()
    return nc


def _bf16(a):
    return np.ascontiguousarray(a).astype(ml_dtypes.bfloat16)


def _in_maps(inputs):
    q = np.asarray(inputs["query"], dtype=np.float32)
    k = np.asarray(inputs["key_in"], dtype=np.float32)
    v = np.asarray(inputs["value"], dtype=np.float32)
    Wq = np.asarray(inputs["Wq"], dtype=np.float32)
    Wk = np.asarray(inputs["Wk"], dtype=np.float32)
    Wv = np.asarray(inputs["Wv"], dtype=np.float32)
    Wo = np.asarray(inputs["Wo"], dtype=np.float32)
    bq = np.asarray(inputs["bq"], dtype=np.float32)
    bk = np.asarray(inputs["bk"], dtype=np.float32)
    bv = np.asarray(inputs["bv"], dtype=np.float32)

    xT = [[_bf16(x[b].T) for b in range(B)] for x in (q, k, v)]
    maps = []
    for c in range(NCORES):
        b, g = divmod(c, TP)
        sl = slice(g * CL, (g + 1) * CL)
        maps.append({
            "xqT": xT[0][b], "xkT": xT[1][b], "xvT": xT[2][b],
            "wqT": _bf16(Wq[sl, :].T), "wkT": _bf16(Wk[sl, :].T),
            "wvT": _bf16(Wv[sl, :].T), "woT": _bf16(Wo[:, sl].T),
            "bq": np.ascontiguousarray(bq[sl]),
            "bk": np.ascontiguousarray(bk[sl]),
            "bv": np.ascontiguousarray(bv[sl]),
        })
    return maps


TRACE = False
TMPDIR = None
LAST_RESULT = None



def kernel(**inputs):
    global _NC, LAST_RESULT
    from concourse.bass_utils import run_bass_kernel_spmd

    if _NC is None:
        _NC = _build_nc()
    maps = _in_maps(inputs)
    res = run_bass_kernel_spmd(_NC, maps, core_ids=list(range(NCORES)),
                               trace=TRACE, tmpdir=TMPDIR)
    LAST_RESULT = res

    bo = np.asarray(inputs["bo"], dtype=np.float32)
    out = np.zeros((B, S, D), dtype=np.float32)
    for c in range(NCORES):
        b, _ = divmod(c, TP)
        out[b] += res.results[c]["zT"].astype(np.float32).T
    out += bo[None, None, :]
    return out
